# revision 9
# baseline (speedup 1.0000x reference)
"""Trainium2 Bass kernel for nn_AttentionSup (dense transformer attention block).

Computation (see reference):
  qkv = x @ W_qkv; per-head attention softmax(q k^T / sqrt(d)) v;
  domain-gate (tiny MLP + softmax over heads) multiplies the attention
  output per (batch, head, dim); out = gated @ W_out + b_out.

Sharding over 8 NeuronCores: (batch b in 0..3) x (head-group g in 0..1),
4 heads per core - data-parallel over batch, tensor-parallel over heads.
Each core computes a partial output [2048, 512] for its batch from its 4
heads; the host sums the two head-group partials per batch and adds b_out.
The tiny domain-gate MLP is computed on the host and folded into the V
projection weights (it scales O columns, i.e. W_v columns).

Kernel structure (v2, rewritten from trace analysis of v1):
- The hard floor is the ACT (scalar) engine exp stream: 16.8M exps/core at
  ~0.83ns/elem + per-instruction overhead. Exp tiles are [128,1536] (3 PSUM
  banks), the largest that lets two S tiles double-buffer in PSUM.
- Scores are computed in fp32r chunks [128k, 512q] (measured 227ns dense).
- PV uses the FLIPPED orientation: lhsT = P^T chunk [128k, 128q] (bf16 from
  the exp), rhs = V_ext [128k, 65] (bf16, ones column for row sums), out
  O[q,65] accumulated in PSUM. Measured 34ns/matmul vs 216ns for the
  unflipped [65,512] form - weight loads overlap fully on TRN2's PE.
- Softmax normalization is a per-partition reciprocal + one broadcast
  multiply on DVE (rsum lands in O's 65th column).
- og [q,f] is transposed back to [f,n] with PE transpose instructions
  (71ns/tile) so the final projection runs as plain fp32r matmuls.
- PSUM: 2x S[128,1536] (6 banks) + O-slot bank (7 packed 65-wide slots,
  rotating) + 1 staging bank for projection chunks/transposes/finals.
- Phase 1 (QKV projections) is DMA-gated: x rides 3 DMA queues, weights a
  4th; k0/q0 projections start dt-outer as chunks land; the attention
  stream starts right after q0 drains while v/k1/q1 still project.
"""

import sys

sys.path.insert(0, "/opt/trn_rl_repo")

import numpy as np
from contextlib import ExitStack

import concourse.bass as bass
import concourse.tile as tile
from concourse import bacc, mybir
from concourse.bass_utils import run_bass_kernel_spmd


def _install_ntff_hook():
    """Provide antenv.axon_hooks (absent from the image) so
    run_bass_kernel_spmd(trace=True) can capture NTFF profiles under axon."""
    import types

    if "antenv.axon_hooks" in sys.modules:
        return
    mod = types.ModuleType("antenv.axon_hooks")
    mod._HOOK = None
    mod.set_axon_ntff_profile_hook = lambda h: setattr(mod, "_HOOK", h)
    mod.get_axon_ntff_profile_hook = lambda: mod._HOOK
    try:
        from trn_agent_boot.trn_boot import _ntff_profile_via_ctypes

        mod._HOOK = _ntff_profile_via_ctypes("/opt/axon/libaxon_pjrt.so")
    except Exception:
        pass
    sys.modules["antenv.axon_hooks"] = mod
    try:
        import antenv

        antenv.axon_hooks = mod
    except Exception:
        pass


_install_ntff_hook()

f32 = mybir.dt.float32
f32r = mybir.dt.float32r
bf16 = mybir.dt.bfloat16
Exp = mybir.ActivationFunctionType.Exp
Copy = mybir.ActivationFunctionType.Copy

# Problem shapes (hardcoded per contract)
B, N, D = 4, 2048, 512
HEADS, DH = 8, 64
INNER = HEADS * DH  # 512
SCALE = DH**-0.5
NCORES = 8
HG = 2  # head groups (tensor-parallel degree)
HPC = HEADS // HG  # 4 heads per core
F = HPC * DH  # 256 inner dims per core
NT = N // 128  # 16 token tiles
DT = D // 4 // 128 * 4  # 4 d-tiles
GRP = 3  # score chunks per exp tile
PV_LAG = 2  # groups between exp emission and its PV consumption
FILL_SC = 2  # duplicated score-chunk matmuls per group (PE clock keep-warm)

_NC_CACHE = {}


def _build(dbg=False):
    nc = bacc.Bacc("TRN2", target_bir_lowering=False, debug=False, num_devices=NCORES)

    xT_d = nc.dram_tensor("xT", [D, N], f32, kind="ExternalInput")
    wq_d = nc.dram_tensor("wq", [D, F], f32, kind="ExternalInput")
    wk_d = nc.dram_tensor("wk", [D, F], f32, kind="ExternalInput")
    wv_d = nc.dram_tensor("wv", [D, F], f32, kind="ExternalInput")  # gate-scaled
    wo_d = nc.dram_tensor("wo", [F, D], f32, kind="ExternalInput")
    id_d = nc.dram_tensor("ident", [128, 128], f32, kind="ExternalInput")
    part_d = nc.dram_tensor("part", [N, D], f32, kind="ExternalOutput")
    if dbg:
        dbg_d = {
            "d_qt0": nc.dram_tensor("d_qt0", [128, N], f32, kind="ExternalOutput"),
            "d_kt0": nc.dram_tensor("d_kt0", [128, N], f32, kind="ExternalOutput"),
            "d_v": nc.dram_tensor("d_v", [128, NT * HPC * 65], f32, kind="ExternalOutput"),
            "d_og": nc.dram_tensor("d_og", [128, NT * F], f32, kind="ExternalOutput"),
            "d_ogT": nc.dram_tensor("d_ogT", [128, 2 * N], f32, kind="ExternalOutput"),
        }

    with tile.TileContext(nc) as tc:
        with ExitStack() as ctx:
            persist = ctx.enter_context(tc.tile_pool(name="persist", bufs=1))
            ptA = ctx.enter_context(tc.tile_pool(name="ptA", bufs=3))
            ptB = ctx.enter_context(tc.tile_pool(name="ptB", bufs=3))
            normp = ctx.enter_context(tc.tile_pool(name="normp", bufs=4))
            obp = ctx.enter_context(tc.tile_pool(name="obp", bufs=2))
            psA = ctx.enter_context(tc.tile_pool(name="psA", bufs=1, space="PSUM"))
            psB = ctx.enter_context(tc.tile_pool(name="psB", bufs=1, space="PSUM"))
            psO = ctx.enter_context(tc.tile_pool(name="psO", bufs=1, space="PSUM"))
            psF = ctx.enter_context(tc.tile_pool(name="psF", bufs=1, space="PSUM"))

            qt = [persist.tile([128, N], f32r, tag=f"qt{i}", name=f"qt{i}") for i in range(2)]
            kt = [persist.tile([128, N], f32r, tag=f"kt{i}", name=f"kt{i}") for i in range(2)]
            v_sb = persist.tile([128, NT, HPC, 65], bf16, tag="v", name="v")
            og = persist.tile([128, NT, F], f32r, tag="og", name="og")
            ogT = persist.tile([128, 2, N], f32r, tag="ogT", name="ogT")
            wo_sb = persist.tile([128, 2, D], f32r, tag="wo", name="wo")
            id_sb = persist.tile([128, 128], f32r, tag="id", name="id")
            warm = persist.tile([1, 64], f32, tag="warm", name="warm")
            warm2 = persist.tile([1, 64], f32, tag="warm2", name="warm2")
            xt = [persist.tile([128, N], f32r, tag=f"xt{d}", name=f"xt{d}") for d in range(DT)]
            w_sb = {
                w: [persist.tile([128, F], f32r, tag=f"{w}{d}", name=f"{w}{d}") for d in range(DT)]
                for w in ("wq", "wk", "wv")
            }

            # ---- Phase 0: DMAs + exp table warm ----
            nc.vector.memset(warm[:], 0.0)
            nc.scalar.activation(warm2[:], warm[:], Exp, scale=0.0)
            nc.vector.memset(v_sb[:, :, :, 64], 1.0)

            # x across the three DMA-capable queues (sync / gpsimd / scalar);
            # scalar first carries the k/q weights it needs earliest.
            xr = xT_d[:].rearrange("(dt p) n -> p dt n", p=128).bitcast(f32r)
            nc.sync.dma_start(xt[0][:], xr[:, 0])
            nc.gpsimd.dma_start(xt[1][:], xr[:, 1])
            nc.sync.dma_start(xt[2][:], xr[:, 2])
            for wname, w_d in (("wk", wk_d), ("wq", wq_d)):
                wr = w_d[:].rearrange("(dt p) f -> p dt f", p=128).bitcast(f32r)
                for d in range(DT):
                    nc.scalar.dma_start(w_sb[wname][d][:], wr[:, d])
            nc.scalar.dma_start(xt[3][:], xr[:, 3])
            wr = wv_d[:].rearrange("(dt p) f -> p dt f", p=128).bitcast(f32r)
            for d in range(DT):
                nc.gpsimd.dma_start(w_sb["wv"][d][:], wr[:, d])
            nc.scalar.dma_start(
                wo_sb[:], wo_d[:].rearrange("(ft p) m -> p ft m", p=128).bitcast(f32r)
            )
            nc.scalar.dma_start(id_sb[:], id_d[:].bitcast(f32r))

            # ---- Phase 1: QKV projections (serial, dt-outer, all 8 banks) ----
            drain_alt = [0]

            def drain(dst_ap, src_ap):
                # alternate ACT / DVE for projection-chunk drains
                if drain_alt[0] % 2 == 0:
                    nc.scalar.activation(dst_ap, src_ap, Copy)
                else:
                    nc.vector.tensor_copy(dst_ap, src_ap)
                drain_alt[0] += 1

            def proj_qk(wname, hp, dst, psums):
                for d in range(DT):
                    for c in range(4):
                        nc.tensor.matmul(
                            psums[c],
                            w_sb[wname][d][:, hp * 128 : (hp + 1) * 128],
                            xt[d][:, c * 512 : (c + 1) * 512],
                            start=(d == 0),
                            stop=(d == DT - 1),
                        )
                for c in range(4):
                    drain(dst[:, c * 512 : (c + 1) * 512], psums[c])

            sA = psA.tile([128, 1536], f32, tag="S", name="ph1_a")
            sB = psB.tile([128, 1536], f32, tag="S", name="ph1_b")
            sO = psO.tile([128, 512], f32, tag="O", name="ph1_o")
            sF = psF.tile([128, 512], f32, tag="Fp", name="ph1_f")
            # k0+q0 jointly dt-outer so a late x chunk doesn't serialize them
            kch = [sA[:, 0:512], sA[:, 512:1024], sA[:, 1024:1536], sO[:]]
            qch = [sB[:, 0:512], sB[:, 512:1024], sB[:, 1024:1536], sF[:]]
            for d in range(DT):
                for chs, wname, hp in ((kch, "wk", 0), (qch, "wq", 0)):
                    for c in range(4):
                        nc.tensor.matmul(
                            chs[c],
                            w_sb[wname][d][:, hp * 128 : (hp + 1) * 128],
                            xt[d][:, c * 512 : (c + 1) * 512],
                            start=(d == 0),
                            stop=(d == DT - 1),
                        )
            for c in range(4):
                drain(kt[0][:, c * 512 : (c + 1) * 512], kch[c])
                drain(qt[0][:, c * 512 : (c + 1) * 512], qch[c])

            # V projection: 16 chunks [128n, 256f], dt-outer, packed 2/bank
            sA = psA.tile([128, 1536], f32, tag="S", name="ph1v_a")
            sB = psB.tile([128, 1536], f32, tag="S", name="ph1v_b")
            sO = psO.tile([128, 512], f32, tag="O", name="ph1v_o")
            sF = psF.tile([128, 512], f32, tag="Fp", name="ph1v_f")
            vch = [sA[:, i * 256 : (i + 1) * 256] for i in range(6)]
            vch += [sB[:, i * 256 : (i + 1) * 256] for i in range(6)]
            vch += [sO[:, 0:256], sO[:, 256:512], sF[:, 0:256], sF[:, 256:512]]
            for d in range(DT):
                for k in range(NT):
                    # two 256-wide chunks share each PSUM bank: start marks the
                    # whole bank pending-zero, so only the bank's first write
                    # starts and only its last stops
                    nc.tensor.matmul(
                        vch[k],
                        xt[d][:, k * 128 : (k + 1) * 128],
                        w_sb["wv"][d][:],
                        start=(d == 0 and k % 2 == 0),
                        stop=(d == DT - 1 and k % 2 == 1),
                    )
            for k in range(NT):
                drain(
                    v_sb[:, k, :, 0:64],
                    vch[k].rearrange("p (h e) -> p h e", e=64),
                )

            # second head-pair projections
            sA = psA.tile([128, 1536], f32, tag="S", name="ph1_a2")
            sB = psB.tile([128, 1536], f32, tag="S", name="ph1_b2")
            sO = psO.tile([128, 512], f32, tag="O", name="ph1_o2")
            sF = psF.tile([128, 512], f32, tag="Fp", name="ph1_f2")
            proj_qk("wk", 1, kt[1], [sA[:, 0:512], sA[:, 512:1024], sA[:, 1024:1536], sO[:]])
            proj_qk("wq", 1, qt[1], [sB[:, 0:512], sB[:, 512:1024], sB[:, 1024:1536], sF[:]])

            # ---- Phase 2: attention stream ----
            chunks = [(qc, h, k) for qc in range(4) for h in range(HPC) for k in range(NT)]
            groups = [chunks[i * GRP : (i + 1) * GRP] for i in range((len(chunks) + GRP - 1) // GRP)]

            o_cur = [None]  # per-block psO generation
            psf_work = []  # deferred transpose/final generators
            group_meta = []  # per group: (pt tile, [(qc,h,k,ci), ...])

            def emit_scores(gi, grp):
                pool = psA if gi % 2 == 0 else psB
                S = pool.tile([128, 1536], f32, tag="S", name=f"s{gi}")
                for ci, (qc, h, k) in enumerate(grp):
                    hp, h01 = h // 2, (h % 2) * 64
                    # keep-warm: duplicated score matmuls (overwritten by the
                    # real one below) keep the PE dense so it holds 2.4 GHz
                    reps = 2 if ci < FILL_SC else 1
                    for _ in range(reps):
                        nc.tensor.matmul(
                            S[:, ci * 512 : (ci + 1) * 512],
                            kt[hp][h01 : h01 + 64, k * 128 : (k + 1) * 128],
                            qt[hp][h01 : h01 + 64, qc * 512 : (qc + 1) * 512],
                            start=True,
                            stop=True,
                        )
                ptp = ptA if gi % 2 == 0 else ptB
                pt = ptp.tile([128, 1536], bf16, tag="pt", name=f"pt{gi}")
                wid = 512 * len(grp)
                nc.scalar.activation(pt[:, 0:wid], S[:, 0:wid], Exp, scale=SCALE)
                group_meta.append((pt, list(grp)))

            def emit_normalize(qc, h):
                o_ps = o_cur[0]
                o_slots = o_ps[:].rearrange("p (s c) -> p s c", c=128)
                rv = normp.tile([128, 4], f32, tag="rv", name=f"rv{qc}{h}")
                nc.vector.reciprocal_approx_fast(rv[:], o_slots[:, :, 64])
                # og[:, qc*4+j, h*64:(h+1)*64] = O[:, slot j, 0:64] * rv[j]
                for j in range(4):
                    nc.vector.tensor_scalar(
                        og[:, qc * 4 + j, h * 64 : (h + 1) * 64],
                        o_slots[:, j, 0:64],
                        rv[:, j : j + 1],
                        None,
                        mybir.AluOpType.mult,
                    )

            def emit_pv_group(g):
                pt, grp = group_meta[g]
                for ci, (qc, h, k) in enumerate(grp):
                    if k == 0:
                        o_cur[0] = psO.tile([128, 512], f32, tag="O", name=f"o{qc}_{h}")
                    o_ps = o_cur[0]
                    for j in range(4):
                        # slots j*128..j*128+65 share one bank: bank-level
                        # start on the block's first write, stop on its last
                        nc.tensor.matmul(
                            o_ps[:, j * 128 : j * 128 + 65],
                            pt[:, ci * 512 + j * 128 : ci * 512 + (j + 1) * 128],
                            v_sb[:, k, h, :],
                            start=(k == 0 and j == 0),
                            stop=(k == NT - 1 and j == 3),
                        )
                    if k == NT - 1:
                        emit_normalize(qc, h)
                        if h == HPC - 1:
                            psf_work.append(("T", qc, 0))
                            psf_work.append(("T", qc, 1))
                            for nt in range(qc * 4, qc * 4 + 4):
                                psf_work.append(("F", nt, 0))

            def emit_psf(item):
                kind, a, b = item
                if kind == "T":
                    qc, ft = a, b
                    tp = psF.tile([128, 512], f32, tag="Fp", name=f"t{qc}{ft}")
                    tpr = tp[:].bitcast(f32r)
                    for j in range(4):
                        nc.tensor.transpose(
                            tpr[:, j * 128 : (j + 1) * 128],
                            og[:, qc * 4 + j, ft * 128 : (ft + 1) * 128],
                            id_sb[:],
                        )
                    nc.vector.tensor_copy(
                        ogT[:, ft, qc * 512 : (qc + 1) * 512], tp[:]
                    )
                else:
                    nt = a
                    fp = psF.tile([128, 512], f32, tag="Fp", name=f"f{nt}")
                    for ft in range(2):
                        nc.tensor.matmul(
                            fp[:],
                            ogT[:, ft, nt * 128 : (nt + 1) * 128],
                            wo_sb[:, ft, :],
                            start=(ft == 0),
                            stop=(ft == 1),
                        )
                    ob = obp.tile([128, 512], f32, tag="ob", name=f"ob{nt}")
                    nc.vector.tensor_copy(ob[:], fp[:])
                    nc.sync.dma_start(part_d[nt * 128 : (nt + 1) * 128, :], ob[:])

            for gi, grp in enumerate(groups):
                emit_scores(gi, grp)
                if gi - PV_LAG >= 0:
                    emit_pv_group(gi - PV_LAG)
                if psf_work:
                    emit_psf(psf_work.pop(0))
            for g in range(len(groups) - PV_LAG, len(groups)):
                emit_pv_group(g)
            while psf_work:
                emit_psf(psf_work.pop(0))

            if dbg:
                dv = persist.tile([128, NT * HPC * 65], f32, tag="dv", name="dv")
                nc.vector.tensor_copy(dv[:], v_sb[:].rearrange("p a b c -> p (a b c)"))
                nc.sync.dma_start(dbg_d["d_qt0"][:], qt[0][:].bitcast(f32))
                nc.sync.dma_start(dbg_d["d_kt0"][:], kt[0][:].bitcast(f32))
                nc.sync.dma_start(dbg_d["d_v"][:], dv[:])
                nc.sync.dma_start(dbg_d["d_og"][:], og[:].rearrange("p a b -> p (a b)").bitcast(f32))
                nc.sync.dma_start(dbg_d["d_ogT"][:], ogT[:].rearrange("p a b -> p (a b)").bitcast(f32))

    nc.compile()
    return nc


def _get_nc(dbg=False):
    key = "nc_dbg" if dbg else "nc"
    if key not in _NC_CACHE:
        _NC_CACHE[key] = _build(dbg)
    return _NC_CACHE[key]


def _prepare_in_maps(x, domain_label, W_qkv, W_d1, b_d1, W_d2, b_d2, W_out, b_out):
    x = np.asarray(x, np.float32)
    domain_label = np.asarray(domain_label, np.float32)
    W_qkv = np.asarray(W_qkv, np.float32)
    W_d1 = np.asarray(W_d1, np.float32)
    b_d1 = np.asarray(b_d1, np.float32)
    W_d2 = np.asarray(W_d2, np.float32)
    b_d2 = np.asarray(b_d2, np.float32)
    W_out = np.asarray(W_out, np.float32)

    # host: domain gate MLP + softmax over heads (tiny)
    d1 = np.maximum(domain_label @ W_d1 + b_d1, 0.0)
    d = d1 @ W_d2 + b_d2  # [B, INNER]
    d = d.reshape(B, HEADS, DH)
    e = np.exp(d - d.max(axis=1, keepdims=True))
    gate = (e / e.sum(axis=1, keepdims=True)).reshape(B, INNER).astype(np.float32)

    ident = np.eye(128, dtype=np.float32)
    in_maps = []
    for c in range(NCORES):
        b, g = c // HG, c % HG
        sl = slice(g * F, (g + 1) * F)
        in_maps.append(
            {
                "xT": np.ascontiguousarray(x[b].T),
                "wq": np.ascontiguousarray(W_qkv[:, sl]),
                "wk": np.ascontiguousarray(W_qkv[:, INNER:][:, sl]),
                "wv": np.ascontiguousarray(
                    W_qkv[:, 2 * INNER :][:, sl] * gate[b, sl][None, :]
                ),
                "wo": np.ascontiguousarray(W_out[sl, :]),
                "ident": ident,
            }
        )
    return in_maps


def _run(in_maps, trace=False, tmpdir=None, dbg=False):
    nc = _get_nc(dbg)
    return run_bass_kernel_spmd(
        nc, in_maps, list(range(NCORES)), trace=trace, tmpdir=tmpdir
    )


def _assemble(results, b_out):
    b_out = np.asarray(b_out, np.float32)
    out = np.empty((B, N, D), np.float32)
    for b in range(B):
        out[b] = results[HG * b]["part"] + results[HG * b + 1]["part"] + b_out
    return out


def kernel(x, domain_label, W_qkv, W_d1, b_d1, W_d2, b_d2, W_out, b_out):
    in_maps = _prepare_in_maps(
        x, domain_label, W_qkv, W_d1, b_d1, W_d2, b_d2, W_out, b_out
    )
    res = _run(in_maps, trace=False)
    return _assemble(res.results, b_out)


# revision 13
# speedup vs baseline: 1.2828x; 1.2828x over previous
"""Trainium2 Bass kernel for nn_AttentionSup (dense transformer attention block).

Computation (see reference):
  qkv = x @ W_qkv; per-head attention softmax(q k^T / sqrt(d)) v;
  domain-gate (tiny MLP + softmax over heads) multiplies the attention
  output per (batch, head, dim); out = gated @ W_out + b_out.

Sharding over 8 NeuronCores: (batch b in 0..3) x (head-group g in 0..1),
4 heads per core - data-parallel over batch, tensor-parallel over heads.
Each core computes a partial output [2048, 512] for its batch from its 4
heads; the host sums the two head-group partials per batch and adds b_out.
The tiny domain-gate MLP is computed on the host and folded into the V
projection weights (it scales O columns, i.e. W_v columns).

Kernel structure (v2, rewritten from trace analysis of v1):
- The hard floor is the ACT (scalar) engine exp stream: 16.8M exps/core at
  ~0.83ns/elem + per-instruction overhead. Exp tiles are [128,1536] (3 PSUM
  banks), the largest that lets two S tiles double-buffer in PSUM.
- Scores are computed in fp32r chunks [128k, 512q] (measured 227ns dense).
- PV uses the FLIPPED orientation: lhsT = P^T chunk [128k, 128q] (bf16 from
  the exp), rhs = V_ext [128k, 65] (bf16, ones column for row sums), out
  O[q,65] accumulated in PSUM. Measured 34ns/matmul vs 216ns for the
  unflipped [65,512] form - weight loads overlap fully on TRN2's PE.
- Softmax normalization is a per-partition reciprocal + one broadcast
  multiply on DVE (rsum lands in O's 65th column).
- og [q,f] is transposed back to [f,n] with PE transpose instructions
  (71ns/tile) so the final projection runs as plain fp32r matmuls.
- PSUM: 2x S[128,1536] (6 banks) + O-slot bank (7 packed 65-wide slots,
  rotating) + 1 staging bank for projection chunks/transposes/finals.
- Phase 1 (QKV projections) is DMA-gated: x rides 3 DMA queues, weights a
  4th; k0/q0 projections start dt-outer as chunks land; the attention
  stream starts right after q0 drains while v/k1/q1 still project.
"""

import sys

sys.path.insert(0, "/opt/trn_rl_repo")

import numpy as np
from contextlib import ExitStack

import concourse.bass as bass
import concourse.tile as tile
from concourse import bacc, mybir
from concourse.bass_utils import run_bass_kernel_spmd


def _install_ntff_hook():
    """Provide antenv.axon_hooks (absent from the image) so
    run_bass_kernel_spmd(trace=True) can capture NTFF profiles under axon."""
    import types

    if "antenv.axon_hooks" in sys.modules:
        return
    mod = types.ModuleType("antenv.axon_hooks")
    mod._HOOK = None
    mod.set_axon_ntff_profile_hook = lambda h: setattr(mod, "_HOOK", h)
    mod.get_axon_ntff_profile_hook = lambda: mod._HOOK
    try:
        from trn_agent_boot.trn_boot import _ntff_profile_via_ctypes

        mod._HOOK = _ntff_profile_via_ctypes("/opt/axon/libaxon_pjrt.so")
    except Exception:
        pass
    sys.modules["antenv.axon_hooks"] = mod
    try:
        import antenv

        antenv.axon_hooks = mod
    except Exception:
        pass


_install_ntff_hook()

f32 = mybir.dt.float32
f32r = mybir.dt.float32r
bf16 = mybir.dt.bfloat16
Exp = mybir.ActivationFunctionType.Exp
Copy = mybir.ActivationFunctionType.Copy

# Problem shapes (hardcoded per contract)
B, N, D = 4, 2048, 512
HEADS, DH = 8, 64
INNER = HEADS * DH  # 512
SCALE = DH**-0.5
NCORES = 8
HG = 2  # head groups (tensor-parallel degree)
HPC = HEADS // HG  # 4 heads per core
F = HPC * DH  # 256 inner dims per core
NT = N // 128  # 16 token tiles
DT = D // 4 // 128 * 4  # 4 d-tiles
GRP = 3  # score chunks per exp tile
PV_LAG = 2  # groups between exp emission and its PV consumption
FILL_SC = 0  # duplicated score-chunk matmuls per group (PE clock keep-warm)
OFF_EVERY = 4  # every Nth exp group computed on DVE via Schraudolph bit trick
# exp(z) ~ bitcast_f32(int32(z*SA + SB)): SA = 2^23*log2(e)*SCALE folded,
# SB = 127*2^23 - C with Schraudolph's C minimizing relative error
SA = 12102203.1616 * SCALE
SB = 1065353216.0 - 486411.0 + 32768.0

_NC_CACHE = {}


def _build(dbg=False):
    nc = bacc.Bacc("TRN2", target_bir_lowering=False, debug=False, num_devices=NCORES)

    xT_d = nc.dram_tensor("xT", [D, N], f32, kind="ExternalInput")
    wq_d = nc.dram_tensor("wq", [D, F], f32, kind="ExternalInput")
    wk_d = nc.dram_tensor("wk", [D, F], f32, kind="ExternalInput")
    wv_d = nc.dram_tensor("wv", [D, F], f32, kind="ExternalInput")  # gate-scaled
    wo_d = nc.dram_tensor("wo", [F, D], f32, kind="ExternalInput")
    id_d = nc.dram_tensor("ident", [128, 128], f32, kind="ExternalInput")
    part_d = nc.dram_tensor("part", [N, D], f32, kind="ExternalOutput")
    if dbg:
        dbg_d = {
            "d_qt0": nc.dram_tensor("d_qt0", [128, N], f32, kind="ExternalOutput"),
            "d_kt0": nc.dram_tensor("d_kt0", [128, N], f32, kind="ExternalOutput"),
            "d_v": nc.dram_tensor("d_v", [128, NT * HPC * 65], f32, kind="ExternalOutput"),
            "d_og": nc.dram_tensor("d_og", [128, NT * F], f32, kind="ExternalOutput"),
            "d_ogT": nc.dram_tensor("d_ogT", [128, 2 * N], f32, kind="ExternalOutput"),
        }

    with tile.TileContext(nc) as tc:
        with ExitStack() as ctx:
            persist = ctx.enter_context(tc.tile_pool(name="persist", bufs=1))
            ptA = ctx.enter_context(tc.tile_pool(name="ptA", bufs=3))
            ptB = ctx.enter_context(tc.tile_pool(name="ptB", bufs=3))
            normp = ctx.enter_context(tc.tile_pool(name="normp", bufs=4))
            obp = ctx.enter_context(tc.tile_pool(name="obp", bufs=2))
            psA = ctx.enter_context(tc.tile_pool(name="psA", bufs=1, space="PSUM"))
            psB = ctx.enter_context(tc.tile_pool(name="psB", bufs=1, space="PSUM"))
            psO = ctx.enter_context(tc.tile_pool(name="psO", bufs=1, space="PSUM"))
            psF = ctx.enter_context(tc.tile_pool(name="psF", bufs=1, space="PSUM"))

            qt = [persist.tile([128, N], f32r, tag=f"qt{i}", name=f"qt{i}") for i in range(2)]
            kt = [persist.tile([128, N], f32r, tag=f"kt{i}", name=f"kt{i}") for i in range(2)]
            v_sb = persist.tile([128, NT, HPC, 65], bf16, tag="v", name="v")
            og = persist.tile([128, NT, F], f32r, tag="og", name="og")
            ogT = persist.tile([128, 2, N], f32r, tag="ogT", name="ogT")
            wo_sb = persist.tile([128, 2, D], f32r, tag="wo", name="wo")
            id_sb = persist.tile([128, 128], f32r, tag="id", name="id")
            warm = persist.tile([1, 64], f32, tag="warm", name="warm")
            warm2 = persist.tile([1, 64], f32, tag="warm2", name="warm2")
            xt = [persist.tile([128, N], f32r, tag=f"xt{d}", name=f"xt{d}") for d in range(DT)]
            w_sb = {
                w: [persist.tile([128, F], f32r, tag=f"{w}{d}", name=f"{w}{d}") for d in range(DT)]
                for w in ("wq", "wk", "wv")
            }

            # ---- Phase 0: DMAs + exp table warm ----
            nc.vector.memset(warm[:], 0.0)
            nc.scalar.activation(warm2[:], warm[:], Exp, scale=0.0)
            nc.vector.memset(v_sb[:, :, :, 64], 1.0)

            # x across the three DMA-capable queues (sync / gpsimd / scalar);
            # scalar first carries the k/q weights it needs earliest.
            xr = xT_d[:].rearrange("(dt p) n -> p dt n", p=128).bitcast(f32r)
            nc.sync.dma_start(xt[0][:], xr[:, 0])
            nc.gpsimd.dma_start(xt[1][:], xr[:, 1])
            nc.sync.dma_start(xt[2][:], xr[:, 2])
            for wname, w_d in (("wk", wk_d), ("wq", wq_d)):
                wr = w_d[:].rearrange("(dt p) f -> p dt f", p=128).bitcast(f32r)
                for d in range(DT):
                    nc.scalar.dma_start(w_sb[wname][d][:], wr[:, d])
            nc.scalar.dma_start(xt[3][:], xr[:, 3])
            wr = wv_d[:].rearrange("(dt p) f -> p dt f", p=128).bitcast(f32r)
            for d in range(DT):
                nc.gpsimd.dma_start(w_sb["wv"][d][:], wr[:, d])
            nc.scalar.dma_start(
                wo_sb[:], wo_d[:].rearrange("(ft p) m -> p ft m", p=128).bitcast(f32r)
            )
            nc.scalar.dma_start(id_sb[:], id_d[:].bitcast(f32r))

            # ---- Phase 1: QKV projections (serial, dt-outer, all 8 banks) ----
            drain_alt = [0]

            def drain(dst_ap, src_ap):
                # alternate ACT / DVE for projection-chunk drains
                if drain_alt[0] % 2 == 0:
                    nc.scalar.activation(dst_ap, src_ap, Copy)
                else:
                    nc.vector.tensor_copy(dst_ap, src_ap)
                drain_alt[0] += 1

            def proj_qk(wname, hp, dst, psums):
                for d in range(DT):
                    for c in range(4):
                        nc.tensor.matmul(
                            psums[c],
                            w_sb[wname][d][:, hp * 128 : (hp + 1) * 128],
                            xt[d][:, c * 512 : (c + 1) * 512],
                            start=(d == 0),
                            stop=(d == DT - 1),
                        )
                for c in range(4):
                    drain(dst[:, c * 512 : (c + 1) * 512], psums[c])

            sA = psA.tile([128, 1536], f32, tag="S", name="ph1_a")
            sB = psB.tile([128, 1536], f32, tag="S", name="ph1_b")
            sO = psO.tile([128, 512], f32, tag="O", name="ph1_o")
            sF = psF.tile([128, 512], f32, tag="Fp", name="ph1_f")
            # k0+q0 jointly dt-outer so a late x chunk doesn't serialize them
            kch = [sA[:, 0:512], sA[:, 512:1024], sA[:, 1024:1536], sO[:]]
            qch = [sB[:, 0:512], sB[:, 512:1024], sB[:, 1024:1536], sF[:]]
            for d in range(DT):
                for chs, wname, hp in ((kch, "wk", 0), (qch, "wq", 0)):
                    for c in range(4):
                        nc.tensor.matmul(
                            chs[c],
                            w_sb[wname][d][:, hp * 128 : (hp + 1) * 128],
                            xt[d][:, c * 512 : (c + 1) * 512],
                            start=(d == 0),
                            stop=(d == DT - 1),
                        )
            for c in range(4):
                drain(kt[0][:, c * 512 : (c + 1) * 512], kch[c])
                drain(qt[0][:, c * 512 : (c + 1) * 512], qch[c])

            # V projection: 16 chunks [128n, 256f], dt-outer, packed 2/bank
            sA = psA.tile([128, 1536], f32, tag="S", name="ph1v_a")
            sB = psB.tile([128, 1536], f32, tag="S", name="ph1v_b")
            sO = psO.tile([128, 512], f32, tag="O", name="ph1v_o")
            sF = psF.tile([128, 512], f32, tag="Fp", name="ph1v_f")
            vch = [sA[:, i * 256 : (i + 1) * 256] for i in range(6)]
            vch += [sB[:, i * 256 : (i + 1) * 256] for i in range(6)]
            vch += [sO[:, 0:256], sO[:, 256:512], sF[:, 0:256], sF[:, 256:512]]
            for d in range(DT):
                for k in range(NT):
                    # two 256-wide chunks share each PSUM bank: start marks the
                    # whole bank pending-zero, so only the bank's first write
                    # starts and only its last stops
                    nc.tensor.matmul(
                        vch[k],
                        xt[d][:, k * 128 : (k + 1) * 128],
                        w_sb["wv"][d][:],
                        start=(d == 0 and k % 2 == 0),
                        stop=(d == DT - 1 and k % 2 == 1),
                    )
            for k in range(NT):
                drain(
                    v_sb[:, k, :, 0:64],
                    vch[k].rearrange("p (h e) -> p h e", e=64),
                )

            # second head-pair projections
            sA = psA.tile([128, 1536], f32, tag="S", name="ph1_a2")
            sB = psB.tile([128, 1536], f32, tag="S", name="ph1_b2")
            sO = psO.tile([128, 512], f32, tag="O", name="ph1_o2")
            sF = psF.tile([128, 512], f32, tag="Fp", name="ph1_f2")
            proj_qk("wk", 1, kt[1], [sA[:, 0:512], sA[:, 512:1024], sA[:, 1024:1536], sO[:]])
            proj_qk("wq", 1, qt[1], [sB[:, 0:512], sB[:, 512:1024], sB[:, 1024:1536], sF[:]])

            # ---- Phase 2: attention stream ----
            chunks = [(qc, h, k) for qc in range(4) for h in range(HPC) for k in range(NT)]
            groups = [chunks[i * GRP : (i + 1) * GRP] for i in range((len(chunks) + GRP - 1) // GRP)]

            o_cur = [None]  # per-block psO generation
            psf_work = []  # deferred transpose/final generators
            group_meta = []  # per group: (pt tile, [(qc,h,k,ci), ...])

            def emit_scores(gi, grp):
                pool = psA if gi % 2 == 0 else psB
                S = pool.tile([128, 1536], f32, tag="S", name=f"s{gi}")
                for ci, (qc, h, k) in enumerate(grp):
                    hp, h01 = h // 2, (h % 2) * 64
                    # keep-warm: duplicated score matmuls (overwritten by the
                    # real one below) keep the PE dense so it holds 2.4 GHz
                    reps = 2 if ci < FILL_SC else 1
                    for _ in range(reps):
                        nc.tensor.matmul(
                            S[:, ci * 512 : (ci + 1) * 512],
                            kt[hp][h01 : h01 + 64, k * 128 : (k + 1) * 128],
                            qt[hp][h01 : h01 + 64, qc * 512 : (qc + 1) * 512],
                            start=True,
                            stop=True,
                        )
                ptp = ptA if gi % 2 == 0 else ptB
                wid = 512 * len(grp)
                if OFF_EVERY and gi % OFF_EVERY == OFF_EVERY - 2:
                    # DVE Schraudolph exp: int32(s*SA + SB) bitcast as f32r
                    pti = ptp.tile([128, 1536], mybir.dt.int32, tag="pti", name=f"pti{gi}")
                    nc.vector.tensor_scalar(
                        pti[:, 0:wid],
                        S[:, 0:wid],
                        SA,
                        SB,
                        mybir.AluOpType.mult,
                        mybir.AluOpType.add,
                    )
                    pt = (
                        pti[:]
                        .bitcast(bf16)
                        .rearrange("p (n two) -> p n two", two=2)[:, :, 1]
                    )
                else:
                    ptb = ptp.tile([128, 1536], bf16, tag="pt", name=f"pt{gi}")
                    nc.scalar.activation(ptb[:, 0:wid], S[:, 0:wid], Exp, scale=SCALE)
                    pt = ptb[:]
                group_meta.append((pt, list(grp)))

            def emit_normalize(qc, h):
                o_ps = o_cur[0]
                o_slots = o_ps[:].rearrange("p (s c) -> p s c", c=128)
                rv = normp.tile([128, 4], f32, tag="rv", name=f"rv{qc}{h}")
                nc.vector.reciprocal_approx_fast(rv[:], o_slots[:, :, 64])
                # og[:, qc*4+j, h*64:(h+1)*64] = O[:, slot j, 0:64] * rv[j]
                for j in range(4):
                    nc.vector.tensor_scalar(
                        og[:, qc * 4 + j, h * 64 : (h + 1) * 64],
                        o_slots[:, j, 0:64],
                        rv[:, j : j + 1],
                        None,
                        mybir.AluOpType.mult,
                    )

            def emit_pv_group(g):
                pt, grp = group_meta[g]
                for ci, (qc, h, k) in enumerate(grp):
                    if k == 0:
                        o_cur[0] = psO.tile([128, 512], f32, tag="O", name=f"o{qc}_{h}")
                    o_ps = o_cur[0]
                    for j in range(4):
                        # slots j*128..j*128+65 share one bank: bank-level
                        # start on the block's first write, stop on its last
                        nc.tensor.matmul(
                            o_ps[:, j * 128 : j * 128 + 65],
                            pt[:, ci * 512 + j * 128 : ci * 512 + (j + 1) * 128],
                            v_sb[:, k, h, :],
                            start=(k == 0 and j == 0),
                            stop=(k == NT - 1 and j == 3),
                        )
                    if k == NT - 1:
                        emit_normalize(qc, h)
                        if h == HPC - 1:
                            psf_work.append(("T", qc, 0))
                            psf_work.append(("T", qc, 1))
                            for nt in range(qc * 4, qc * 4 + 4):
                                psf_work.append(("F", nt, 0))

            def emit_psf(item):
                kind, a, b = item
                if kind == "T":
                    qc, ft = a, b
                    tp = psF.tile([128, 512], f32, tag="Fp", name=f"t{qc}{ft}")
                    tpr = tp[:].bitcast(f32r)
                    for j in range(4):
                        nc.tensor.transpose(
                            tpr[:, j * 128 : (j + 1) * 128],
                            og[:, qc * 4 + j, ft * 128 : (ft + 1) * 128],
                            id_sb[:],
                        )
                    nc.vector.tensor_copy(
                        ogT[:, ft, qc * 512 : (qc + 1) * 512], tp[:]
                    )
                else:
                    nt = a
                    fp = psF.tile([128, 512], f32, tag="Fp", name=f"f{nt}")
                    for ft in range(2):
                        nc.tensor.matmul(
                            fp[:],
                            ogT[:, ft, nt * 128 : (nt + 1) * 128],
                            wo_sb[:, ft, :],
                            start=(ft == 0),
                            stop=(ft == 1),
                        )
                    ob = obp.tile([128, 512], f32, tag="ob", name=f"ob{nt}")
                    nc.vector.tensor_copy(ob[:], fp[:])
                    nc.sync.dma_start(part_d[nt * 128 : (nt + 1) * 128, :], ob[:])

            for gi, grp in enumerate(groups):
                emit_scores(gi, grp)
                if gi - PV_LAG >= 0:
                    emit_pv_group(gi - PV_LAG)
                if psf_work:
                    emit_psf(psf_work.pop(0))
            for g in range(len(groups) - PV_LAG, len(groups)):
                emit_pv_group(g)
            while psf_work:
                emit_psf(psf_work.pop(0))

            if dbg:
                dv = persist.tile([128, NT * HPC * 65], f32, tag="dv", name="dv")
                nc.vector.tensor_copy(dv[:], v_sb[:].rearrange("p a b c -> p (a b c)"))
                nc.sync.dma_start(dbg_d["d_qt0"][:], qt[0][:].bitcast(f32))
                nc.sync.dma_start(dbg_d["d_kt0"][:], kt[0][:].bitcast(f32))
                nc.sync.dma_start(dbg_d["d_v"][:], dv[:])
                nc.sync.dma_start(dbg_d["d_og"][:], og[:].rearrange("p a b -> p (a b)").bitcast(f32))
                nc.sync.dma_start(dbg_d["d_ogT"][:], ogT[:].rearrange("p a b -> p (a b)").bitcast(f32))

    nc.compile()
    return nc


def _get_nc(dbg=False):
    key = "nc_dbg" if dbg else "nc"
    if key not in _NC_CACHE:
        _NC_CACHE[key] = _build(dbg)
    return _NC_CACHE[key]


def _prepare_in_maps(x, domain_label, W_qkv, W_d1, b_d1, W_d2, b_d2, W_out, b_out):
    x = np.asarray(x, np.float32)
    domain_label = np.asarray(domain_label, np.float32)
    W_qkv = np.asarray(W_qkv, np.float32)
    W_d1 = np.asarray(W_d1, np.float32)
    b_d1 = np.asarray(b_d1, np.float32)
    W_d2 = np.asarray(W_d2, np.float32)
    b_d2 = np.asarray(b_d2, np.float32)
    W_out = np.asarray(W_out, np.float32)

    # host: domain gate MLP + softmax over heads (tiny)
    d1 = np.maximum(domain_label @ W_d1 + b_d1, 0.0)
    d = d1 @ W_d2 + b_d2  # [B, INNER]
    d = d.reshape(B, HEADS, DH)
    e = np.exp(d - d.max(axis=1, keepdims=True))
    gate = (e / e.sum(axis=1, keepdims=True)).reshape(B, INNER).astype(np.float32)

    ident = np.eye(128, dtype=np.float32)
    in_maps = []
    for c in range(NCORES):
        b, g = c // HG, c % HG
        sl = slice(g * F, (g + 1) * F)
        in_maps.append(
            {
                "xT": np.ascontiguousarray(x[b].T),
                "wq": np.ascontiguousarray(W_qkv[:, sl]),
                "wk": np.ascontiguousarray(W_qkv[:, INNER:][:, sl]),
                "wv": np.ascontiguousarray(
                    W_qkv[:, 2 * INNER :][:, sl] * gate[b, sl][None, :]
                ),
                "wo": np.ascontiguousarray(W_out[sl, :]),
                "ident": ident,
            }
        )
    return in_maps


def _run(in_maps, trace=False, tmpdir=None, dbg=False):
    nc = _get_nc(dbg)
    return run_bass_kernel_spmd(
        nc, in_maps, list(range(NCORES)), trace=trace, tmpdir=tmpdir
    )


def _assemble(results, b_out):
    b_out = np.asarray(b_out, np.float32)
    out = np.empty((B, N, D), np.float32)
    for b in range(B):
        out[b] = results[HG * b]["part"] + results[HG * b + 1]["part"] + b_out
    return out


def kernel(x, domain_label, W_qkv, W_d1, b_d1, W_d2, b_d2, W_out, b_out):
    in_maps = _prepare_in_maps(
        x, domain_label, W_qkv, W_d1, b_d1, W_d2, b_d2, W_out, b_out
    )
    res = _run(in_maps, trace=False)
    return _assemble(res.results, b_out)
